# revision 2
# baseline (speedup 1.0000x reference)
"""Trainium2 Bass kernel v4 for HINGCN-GS (2-metapath, 2-layer GNN).

Sharding strategy (8 NeuronCores, data-parallel over the seed batch):
  - B=512 seeds -> 64 per core; index arrays sharded along dim 0.
  - feats/edge_emb tables are sharded ROW-WISE per core: each core's HBM
    holds exactly the table rows its seed shard references, laid out
    feature-major (transposed) in referencing order. This removes the
    SWDGE per-row descriptor bottleneck (~8ns/row on the Q7 cluster,
    ~230us for the 28k rows a core touches) — the row data streams in as
    a handful of bulk regular DMAs at full HBM bandwidth instead.
  - All arithmetic stays on device: neighbor/edge group-means (DVE
    contiguous reduces), all aggregation/edge-update matmuls (fp16, f32
    PSUM), L2 normalization and the final fc head.
  - Weights replicated, packed into one fp16 and one fp32 tile.

Output is produced feature-major [C, BC] per core; host transposes and
concatenates.
"""

import sys

for _p in ("/opt/trn_rl_repo", "/opt/pypackages"):
    if _p not in sys.path:
        sys.path.insert(0, _p)

import numpy as np

import concourse.bass as bass
import concourse.bacc as bacc
import concourse.mybir as mybir
import concourse.tile as tile

F32 = mybir.dt.float32
F16 = mybir.dt.float16
AF = mybir.ActivationFunctionType
AX = mybir.AxisListType
ADD = mybir.AluOpType.add

S = 10
D = 256
E = 64
C = 8
B = 512
NCORES = 8
BC = B // NCORES          # 64 seeds per core
L1 = BC * S               # 640
L2 = BC * S * S           # 6400
KD = D // 128
NF = BC + L1 + L2         # 7104 feats columns per metapath
NE = L1 + L2              # 7040 edge columns per metapath
_N_TILES = ((0, 320), (320, 320))

# offsets into the packed fp16 weight tile (columns)
_WS = {(m, l): (m * 2 + l) * KD * D for m in range(2) for l in range(2)}
_WN = {(m, l): 2048 + (m * 2 + l) * KD * D for m in range(2) for l in range(2)}
_WEDGE = {(m, l): 4096 + (m * 2 + l) * D for m in range(2) for l in range(2)}
_WE = {m: 5120 + m * 5 * E for m in range(2)}
_W16_COLS = 5760
_BE = (0, 1)
_FCW = 2
_FCB = 18


def _build_nc():
    nc = bacc.Bacc(None, target_bir_lowering=False, dynamic_dma_scratch_size=16384)

    f_d = [nc.dram_tensor(f"f{m}", [128, KD, NF], F16, kind="ExternalInput")
           for m in range(2)]
    e_d = [nc.dram_tensor(f"e{m}", [E, NE], F16, kind="ExternalInput")
           for m in range(2)]
    w16_d = nc.dram_tensor("w16", [128, _W16_COLS], F16, kind="ExternalInput")
    w32_d = nc.dram_tensor("w32", [128, 20], F32, kind="ExternalInput")
    out_d = nc.dram_tensor("out", [C, BC], F32, kind="ExternalOutput")

    with tile.TileContext(nc) as tc:
        with (
            tc.tile_pool(name="singles", bufs=1) as singles,
            tc.tile_pool(name="ps_big", bufs=3, space="PSUM") as ps_big,
            tc.tile_pool(name="ps_tiny", bufs=2, space="PSUM") as ps_tiny,
        ):
            # activation/edge tiles; per-(m, ck) DMA slices so the reduce
            # trees can start as soon as each slice lands. Order: mp0 feats
            # first (longest dependent chain), then weights, both edge
            # tiles (small), and mp1 feats last — the mp1 tail is shortest
            # when its edge data is already resident.
            fT0 = singles.tile([128, KD, NF], F16)
            fT1 = singles.tile([128, KD, NF], F16)
            fT = [fT0, fT1]
            eT0 = singles.tile([E, NE], F16)
            eT1 = singles.tile([E, NE], F16)
            eT = [eT0, eT1]
            FS = BC + L1        # f0+f1 section (704 cols)
            nc.sync.dma_start(out=fT0[:, 0, :], in_=f_d[0][:, 0, :])
            w16_t = singles.tile([128, _W16_COLS], F16)
            nc.sync.dma_start(out=w16_t[:, :], in_=w16_d[:, :])
            w32_t = singles.tile([128, 20], F32)
            nc.sync.dma_start(out=w32_t[:, :], in_=w32_d[:, :])
            nc.sync.dma_start(out=fT0[:, 1, :], in_=f_d[0][:, 1, :])
            nc.sync.dma_start(out=eT0[:, :], in_=e_d[0][:, :])
            nc.sync.dma_start(out=fT1[:, 0, :], in_=f_d[1][:, 0, :])
            nc.sync.dma_start(out=fT1[:, 1, 0:FS], in_=f_d[1][:, 1, 0:FS])
            nc.sync.dma_start(out=eT1[:, 0:L1], in_=e_d[1][:, 0:L1])
            nc.sync.dma_start(out=fT1[:, 1, FS:NF], in_=f_d[1][:, 1, FS:NF])
            # e1-of-mp1 last: it has the shortest dependent chain (e1 tree +
            # the accumulation-group stops), so the post-DMA tail is minimal
            nc.sync.dma_start(out=eT1[:, L1:NE], in_=e_d[1][:, L1:NE])

            ones = singles.tile([128, 1], F32)
            nc.vector.memset(ones[:, :], 1.0)
            ones_row = singles.tile([1, C], F32)
            nc.vector.memset(ones_row[:, :], 1.0)

            def wsl(off, kc, osl):
                return w16_t[:, off + kc * D + osl.start:off + kc * D + osl.stop]

            hsumT = singles.tile([128, KD, BC], F32)

            with nc.allow_low_precision(reason="fp16 activations; tol 2e-2"):
                for m in range(2):
                    f0T = fT[m][:, :, 0:BC]
                    f1Tv = fT[m][:, :, BC:BC + L1]
                    f2Tv = fT[m][:, :, BC + L1:NF]
                    e0Tv = eT[m][:, 0:L1]
                    e1Tv = eT[m][:, L1:NE]

                    # ---- group sums ----
                    # level-1 means first (high priority pins them ahead of
                    # the trees in the DVE schedule): they unblock the k=0
                    # and k=1 aggregation matmuls
                    f1mT = singles.tile([128, KD, BC], F16, tag=f"f1mT{m}")
                    e0mT = singles.tile([E, BC], F16, tag=f"e0mT{m}")
                    with tc.high_priority(offset=1000):
                        for ck in range(KD):
                            nc.vector.tensor_reduce(
                                out=f1mT[:, ck, :],
                                in_=f1Tv[:, ck, :].rearrange("p (g s) -> p g s", s=S),
                                axis=AX.X, op=ADD)
                        nc.vector.tensor_reduce(
                            out=e0mT[:, :],
                            in_=e0Tv.rearrange("p (g s) -> p g s", s=S),
                            axis=AX.X, op=ADD)
                    # f2/e1 regions are shipped s-outer ([s=10, g=640]); the
                    # group sum is an in-place binary tree of contiguous
                    # tensor_tensor adds (2 elem/cycle DVE mode). Slot 0
                    # ends up holding the sum.
                    for ck in range(KD):
                        v = f2Tv[:, ck, :].rearrange("p (s g) -> p s g", g=L1)
                        nc.vector.tensor_add(v[:, 0:5, :], v[:, 0:5, :], v[:, 5:10, :])
                        nc.vector.tensor_add(v[:, 0:2, :], v[:, 0:2, :], v[:, 2:4, :])
                        nc.vector.tensor_add(v[:, 0:1, :], v[:, 0:1, :], v[:, 1:2, :])
                        nc.vector.tensor_add(v[:, 0:1, :], v[:, 0:1, :], v[:, 4:5, :])
                    f2mT = fT[m][:, :, BC + L1:BC + L1 + L1]     # slot 0
                    ve = e1Tv.rearrange("p (s g) -> p s g", g=L1)
                    nc.vector.tensor_add(ve[:, 0:5, :], ve[:, 0:5, :], ve[:, 5:10, :])
                    nc.vector.tensor_add(ve[:, 0:2, :], ve[:, 0:2, :], ve[:, 2:4, :])
                    nc.vector.tensor_add(ve[:, 0:1, :], ve[:, 0:1, :], ve[:, 1:2, :])
                    nc.vector.tensor_add(ve[:, 0:1, :], ve[:, 0:1, :], ve[:, 4:5, :])
                    e1mT = eT[m][:, L1:L1 + L1]                  # slot 0

                    # ---- layer 0, agg k=0 ----
                    f0pT = singles.tile([128, KD, BC], F16, tag=f"f0pT{m}")
                    for ck in range(KD):
                        sl = slice(ck * 128, (ck + 1) * 128)
                        ps = ps_tiny.tile([128, BC], F32, tag="ps0")
                        nc.tensor.matmul(ps[:, :], wsl(_WS[m, 0], 0, sl), f0T[:, 0, :], start=True, stop=False)
                        nc.tensor.matmul(ps[:, :], wsl(_WS[m, 0], 1, sl), f0T[:, 1, :], start=False, stop=False)
                        nc.tensor.matmul(ps[:, :], wsl(_WN[m, 0], 0, sl), f1mT[:, 0, :], start=False, stop=False)
                        nc.tensor.matmul(ps[:, :], wsl(_WN[m, 0], 1, sl), f1mT[:, 1, :], start=False, stop=False)
                        nc.tensor.matmul(ps[:, :], w16_t[0:E, _WEDGE[m, 0] + sl.start:_WEDGE[m, 0] + sl.stop], e0mT[:, :], start=False, stop=True)
                        nc.scalar.activation(f0pT[:, ck, :], ps[:, :], AF.Relu)

                    # ---- layer 0, agg k=1 ----
                    f1pT = singles.tile([128, KD, L1], F16, tag=f"f1pT{m}")
                    for ck in range(KD):
                        sl = slice(ck * 128, (ck + 1) * 128)
                        for (n0, nn) in _N_TILES:
                            nsl = slice(n0, n0 + nn)
                            ps = ps_big.tile([128, 320], F32, tag="ps1")
                            nc.tensor.matmul(ps[:, :nn], wsl(_WS[m, 0], 0, sl), f1Tv[:, 0, nsl], start=True, stop=False)
                            nc.tensor.matmul(ps[:, :nn], wsl(_WS[m, 0], 1, sl), f1Tv[:, 1, nsl], start=False, stop=False)
                            nc.tensor.matmul(ps[:, :nn], wsl(_WN[m, 0], 0, sl), f2mT[:, 0, nsl], start=False, stop=False)
                            nc.tensor.matmul(ps[:, :nn], wsl(_WN[m, 0], 1, sl), f2mT[:, 1, nsl], start=False, stop=False)
                            nc.tensor.matmul(ps[:, :nn], w16_t[0:E, _WEDGE[m, 0] + sl.start:_WEDGE[m, 0] + sl.stop], e1mT[:, nsl], start=False, stop=True)
                            # alternate PSUM drains between Scalar and DVE
                            if ck == 0:
                                nc.scalar.activation(f1pT[:, ck, nsl], ps[:, :nn], AF.Relu)
                            else:
                                nc.vector.tensor_scalar_max(f1pT[:, ck, nsl], ps[:, :nn], 0.0)

                    # ---- edge update ----
                    e0pT = singles.tile([E, L1], F16, tag=f"e0pT{m}")
                    wo = _WE[m]
                    for (n0, nn) in _N_TILES:
                        nsl = slice(n0, n0 + nn)
                        g0, gn = n0 // S, nn // S
                        ps = ps_big.tile([128, 320], F32, tag="ps1")
                        src0 = f0pT[:, 0, g0:g0 + gn].unsqueeze(2).to_broadcast([128, gn, S])
                        src1 = f0pT[:, 1, g0:g0 + gn].unsqueeze(2).to_broadcast([128, gn, S])
                        nc.tensor.matmul(ps[:E, :nn], w16_t[:, wo + 0 * E:wo + 1 * E], src0, start=True, stop=False)
                        nc.tensor.matmul(ps[:E, :nn], w16_t[:, wo + 1 * E:wo + 2 * E], src1, start=False, stop=False)
                        nc.tensor.matmul(ps[:E, :nn], w16_t[:, wo + 2 * E:wo + 3 * E], f1pT[:, 0, nsl], start=False, stop=False)
                        nc.tensor.matmul(ps[:E, :nn], w16_t[:, wo + 3 * E:wo + 4 * E], f1pT[:, 1, nsl], start=False, stop=False)
                        nc.tensor.matmul(ps[:E, :nn], w16_t[0:E, wo + 4 * E:wo + 5 * E], e0Tv[:, nsl], start=False, stop=True)
                        nc.scalar.activation(e0pT[:, nsl], ps[:E, :nn], AF.Tanh,
                                             bias=w32_t[0:E, _BE[m]:_BE[m] + 1])
                    if m == 1:
                        # pre-warm the sqrt activation table off the critical
                        # path: the head's Sqrt lives in a different table set
                        # than Tanh, so the switch must happen after the LAST
                        # tanh but before the head. Anchoring the input to the
                        # last tanh's output pins the scheduler; scale=0 /
                        # bias=1 makes the value sqrt(0*x+1)=1, a no-op write
                        # into ones_row (which is read later, so not DCE'd).
                        nc.scalar.activation(ones_row[0:1, 0:1],
                                             e0pT[0:1, L1 - 1:L1], AF.Sqrt,
                                             bias=1.0, scale=0.0)

                    # ---- layer 1 ----
                    f1pmT = singles.tile([128, KD, BC], F16, tag=f"f1pmT{m}")
                    nc.vector.tensor_reduce(
                        out=f1pmT[:, :, :],
                        in_=f1pT[:, :, :].rearrange("p c (g s) -> p c g s", s=S),
                        axis=AX.X, op=ADD)
                    e0pmT = singles.tile([E, BC], F16, tag=f"e0pmT{m}")
                    nc.vector.tensor_reduce(
                        out=e0pmT[:, :],
                        in_=e0pT[:, :].rearrange("p (g s) -> p g s", s=S),
                        axis=AX.X, op=ADD)
                    for ck in range(KD):
                        sl = slice(ck * 128, (ck + 1) * 128)
                        ps = ps_tiny.tile([128, BC], F32, tag="ps0")
                        nc.tensor.matmul(ps[:, :], wsl(_WS[m, 1], 0, sl), f0pT[:, 0, :], start=True, stop=False)
                        nc.tensor.matmul(ps[:, :], wsl(_WS[m, 1], 1, sl), f0pT[:, 1, :], start=False, stop=False)
                        nc.tensor.matmul(ps[:, :], wsl(_WN[m, 1], 0, sl), f1pmT[:, 0, :], start=False, stop=False)
                        nc.tensor.matmul(ps[:, :], wsl(_WN[m, 1], 1, sl), f1pmT[:, 1, :], start=False, stop=False)
                        nc.tensor.matmul(ps[:, :], w16_t[0:E, _WEDGE[m, 1] + sl.start:_WEDGE[m, 1] + sl.stop], e0pmT[:, :], start=False, stop=True)
                        if m == 0:
                            nc.scalar.copy(hsumT[:, ck, :], ps[:, :])
                        else:
                            nc.vector.tensor_add(hsumT[:, ck, :], hsumT[:, ck, :], ps[:, :])

            # ---- head: row-normalize then fc (f32); output [C, BC] ----
            # square on DVE so Scalar's queue is free for the sqrt-table load
            sq = singles.tile([128, KD, BC], F32)
            nc.vector.tensor_mul(sq[:, :, :], hsumT[:, :, :], hsumT[:, :, :])
            ps_n = ps_tiny.tile([1, BC], F32, tag="ps0")
            nc.tensor.matmul(ps_n[:, :], ones[:, :], sq[:, 0, :], start=True, stop=False)
            nc.tensor.matmul(ps_n[:, :], ones[:, :], sq[:, 1, :], start=False, stop=True)
            # fc matmuls are independent of the norm chain: emit them right
            # after the n2 matmuls so the PE isn't stalled behind reciprocal
            ps_y = ps_tiny.tile([C, BC], F32, tag="ps0")
            nc.tensor.matmul(ps_y[:, :], w32_t[:, _FCW + 0 * C:_FCW + 1 * C], hsumT[:, 0, :], start=True, stop=False)
            nc.tensor.matmul(ps_y[:, :], w32_t[:, _FCW + 1 * C:_FCW + 2 * C], hsumT[:, 1, :], start=False, stop=True)
            n2 = singles.tile([1, BC], F32)
            nc.vector.tensor_scalar_max(n2[:, :], ps_n[:, :], 1e-24)
            nrm = singles.tile([1, BC], F32)
            nc.scalar.sqrt(nrm[:, :], n2[:, :])
            rn = singles.tile([1, BC], F32)
            nc.vector.reciprocal(rn[:, :], nrm[:, :])
            ps_rep = ps_tiny.tile([C, BC], F32, tag="ps0")
            nc.tensor.matmul(ps_rep[:, :], ones_row[:, :], rn[:, :], start=True, stop=True)
            rn_rep = singles.tile([C, BC], F32)
            nc.vector.tensor_copy(rn_rep[:, :], ps_rep[:, :])
            y_sc = singles.tile([C, BC], F32)
            nc.vector.tensor_mul(y_sc[:, :], ps_y[:, :], rn_rep[:, :])
            y_b = singles.tile([C, BC], F32)
            nc.scalar.activation(y_b[:, :], y_sc[:, :], AF.Identity,
                                 bias=w32_t[0:C, _FCB:_FCB + 1])
            nc.sync.dma_start(out=out_d[:, :], in_=y_b[:, :])

    return nc


_NC_CACHE = {}


def _get_nc():
    if "nc" not in _NC_CACHE:
        nc = _build_nc()
        nc.compile()
        _NC_CACHE["nc"] = nc
    return _NC_CACHE["nc"]


def _featmajor(tbl16, rows):
    """Gather rows of [N, D'] fp16 table, return feature-major [128, KD', n]."""
    blk = tbl16[rows]                     # [n, D']
    dd = blk.shape[1]
    return np.ascontiguousarray(
        blk.T.reshape(dd // 128 if dd >= 128 else 1, min(dd, 128), -1)
        .transpose(1, 0, 2))


def _prep_in_maps(ids, feats, n00, n01, n10, n11, e00, e01, e10, e11,
                  edge_emb0, edge_emb1, W_self, W_neigh, W_edge, We, be,
                  fc_w, fc_b):
    f32, f16 = np.float32, np.float16
    feats16 = np.asarray(feats, dtype=f32).astype(f16)
    emb16 = [np.asarray(edge_emb0, dtype=f32).astype(f16),
             np.asarray(edge_emb1, dtype=f32).astype(f16)]

    # ---- packed weights (identical for every core) ----
    w16 = np.zeros((128, _W16_COLS), dtype=f16)
    ws_host = np.asarray(W_self, dtype=f32).reshape(2, 2, KD, 128, D).transpose(0, 1, 3, 2, 4)
    wn_host = (np.asarray(W_neigh, dtype=f32) / S).reshape(2, 2, KD, 128, D).transpose(0, 1, 3, 2, 4)
    wedge_host = np.asarray(W_edge, dtype=f32) / S
    for m in range(2):
        for l in range(2):
            w16[:, _WS[m, l]:_WS[m, l] + KD * D] = ws_host[m, l].reshape(128, KD * D)
            w16[:, _WN[m, l]:_WN[m, l] + KD * D] = wn_host[m, l].reshape(128, KD * D)
            w16[:E, _WEDGE[m, l]:_WEDGE[m, l] + D] = wedge_host[m, l]
    we_pad = np.zeros((2, 5 * 128, E), dtype=f32)
    we_pad[:, :2 * D + E, :] = np.asarray(We, dtype=f32)[:, 0]
    we_host = we_pad.reshape(2, 5, 128, E).transpose(0, 2, 1, 3)
    for m in range(2):
        w16[:, _WE[m]:_WE[m] + 5 * E] = we_host[m].reshape(128, 5 * E)

    w32 = np.zeros((128, 20), dtype=f32)
    be_h = np.asarray(be, dtype=f32)[:, 0]
    w32[:E, _BE[0]] = be_h[0]
    w32[:E, _BE[1]] = be_h[1]
    fcw_h = np.asarray(fc_w, dtype=f32).reshape(KD, 128, C).transpose(1, 0, 2)
    w32[:, _FCW:_FCW + KD * C] = fcw_h.reshape(128, KD * C)
    w32[:C, _FCB] = np.asarray(fc_b, dtype=f32)

    ids = np.asarray(ids)
    neigh1 = (np.asarray(n00), np.asarray(n10))
    neigh2 = (np.asarray(n01), np.asarray(n11))
    eidx1 = (np.asarray(e00), np.asarray(e10))
    eidx2 = (np.asarray(e01), np.asarray(e11))

    in_maps = []
    for k in range(NCORES):
        s0 = slice(k * BC, (k + 1) * BC)
        s1 = slice(k * L1, (k + 1) * L1)
        s2 = slice(k * L2, (k + 1) * L2)
        mdict = {"w16": w16, "w32": w32}
        for m in range(2):
            # level-2 rows shipped s-outer ([member, group]) so the on-device
            # group sum is a contiguous tree of adds
            n2so = neigh2[m][s2].reshape(L1, S).T.ravel()
            frows = np.concatenate([ids[s0], neigh1[m][s1], n2so])
            mdict[f"f{m}"] = _featmajor(feats16, frows)          # [128, 2, 7104]
            e2so = eidx2[m][s2].reshape(L1, S).T.ravel()
            erows = np.concatenate([eidx1[m][s1], e2so])
            mdict[f"e{m}"] = np.ascontiguousarray(emb16[m][erows].T)  # [64, 7040]
        in_maps.append(mdict)
    return in_maps


def kernel(**inputs):
    from concourse.bass_utils import run_bass_kernel_spmd

    nc = _get_nc()
    in_maps = _prep_in_maps(**inputs)
    res = run_bass_kernel_spmd(nc, in_maps, core_ids=list(range(NCORES)))
    out = np.concatenate([r["out"].T for r in res.results], axis=0)
    return out.astype(np.float32)
